# revision 64
# baseline (speedup 1.0000x reference)
"""NeRF MLP kernel for Trainium2 (Bass/Tile), 8-core data-parallel over rays.

v3 design — layer-major, weight-stationary, evac-balanced:

- Layout: features on SBUF partitions, rays (512/core) on the free dim; one
  "chunk" = one sample index s for all local rays.  Chunks are processed in
  super-groups of 8, LAYER-major within the group, so each layer's weights
  stay stationary in the PE across 8 back-to-back matmuls (keeps the PE
  dense/warm; the HAM clock-gate stays at 8/8).
- Hidden matmuls are fp8e4 DoubleRow (weights host-scaled by 512,
  activations stored as 32*h): each 256-contraction layer half is one PE
  matmul at N=512.
- ALL biases are pre-accumulated into PSUM: L0/L4 carry [w;b] rows in the
  k=31 encoding matmuls (enc row 30 is sin(pi/2)=1); L1-L7f get packed k=1
  bias matmuls (rows at 32-aligned groups, even/odd chunks use different
  groups so adjacent bias matmuls pack concurrently).  Every PSUM->SBUF
  evacuation is then a single instruction: relu+scale on ScalarE or
  max0+mult on VectorE — the two PSUM-capable engines, which are the
  throughput floor of this kernel (~1 elem/cycle/partition each).
- Encoding uses the EXACT per-sample z (offsets do not cancel): angle
  u = (s+off[s])*DStep + Qb computed per 4-chunk tile on GpSimd (idle
  otherwise), magic-add range reduction on VectorE, one ScalarE Sin.
- Density (L7d) and color (w8f) rows accumulate IN PSUM across each
  super-group via one-hot weight columns — one cheap [32,512] evac per 8
  chunks instead of per-chunk copies.
- Compositing: w_s = exp(-cumsum) difference via bf16 triangular matmuls.
"""

import math
from contextlib import ExitStack

import numpy as np
import ml_dtypes

import concourse.bass as bass
import concourse.mybir as mybir
import concourse.tile as tile
from concourse import bacc

F32 = mybir.dt.float32
BF16 = mybir.dt.bfloat16
FP8 = mybir.dt.float8e4
AF = mybir.ActivationFunctionType
OP = mybir.AluOpType
DR = mybir.MatmulPerfMode.DoubleRow

S = 64          # samples per ray
B_FULL = 4096   # total rays
N_CORES = 8
BL = B_FULL // N_CORES  # rays per core = 512
H = 256
NEAR, FAR = 2.0, 6.0
DELTA = (FAR - NEAR) / S
L_ENC = 5
ENC = 3 * L_ENC * 2  # 30
TWO_PI = 2.0 * math.pi
MAGIC = 12582912.0  # 1.5 * 2**23, fp32 round-to-nearest trick

WS = 512.0      # fp8 weight scale
AS = 32.0       # fp8 activation scale (stored act = 32*h)
PS = WS * AS    # psum scale for biased layers = 16384
INV_WS = 1.0 / WS    # 2^-9: psum -> stored-act scale
INV_PS = 1.0 / PS    # 2^-14: tail evac scale

NPF8 = ml_dtypes.float8_e4m3
NPBF = ml_dtypes.bfloat16

# evac engine per hidden layer (7 = L7f). 'vec' layers use max0+mult
# tensor_scalar; 'act' layers use Relu activation. All read bias from PSUM.
EV = {0: "vec", 1: "vec", 2: "act", 3: "act", 4: "vec", 5: "act", 6: "act",
      7: "vec"}


def host_constants():
    c = {}
    freqs = (2.0 ** (np.arange(L_ENC, dtype=np.float64) - 2)) * math.pi  # [L]
    fturn = np.zeros((ENC, 1), dtype=np.float32)
    phase = np.zeros((ENC, 1), dtype=np.float32)
    for cc in range(3):
        for ll in range(L_ENC):
            for tt in range(2):
                j = cc * (L_ENC * 2) + ll * 2 + tt
                fturn[j, 0] = freqs[ll] / TWO_PI
                phase[j, 0] = 0.0 if tt == 0 else 0.25  # pi/2 in turns
    c["fturn30"] = fturn
    c["phase30"] = phase
    c["cap1e10"] = np.full((1, BL), 1.0e10, dtype=np.float32)
    c["q025"] = np.full((1, BL), 0.25, dtype=np.float32)
    c["svec64"] = (NEAR + np.arange(S, dtype=np.float32)[:, None] * DELTA)
    c["srow64"] = np.arange(S, dtype=np.float32)[:, None]
    c["ltri"] = np.triu(np.ones((S, S), dtype=np.float32)).astype(NPBF)
    c["ltri2"] = (np.triu(np.ones((S, S))) + np.eye(S)).astype(NPBF)
    c["ones31"] = np.ones((3, 1), dtype=np.float32)
    c["ones641"] = np.ones((S, 1), dtype=NPBF)
    return c


def host_weights(inp):
    w = {}

    def kstack(m):  # [256, M] -> [128, 2, M]
        return np.ascontiguousarray(m.reshape(2, 128, m.shape[1]).transpose(1, 0, 2))

    # k=31 encoding matmuls carry weight+bias scaled by PS: 4 identical
    # 32-row blocks (one per chunk slot in a 4-chunk enc tile); row 30 is
    # the bias row (enc row 30 evaluates to 1), row 31 zero.
    def blk4(wmat, bvec, m):
        t = np.zeros((128, 128), dtype=np.float32)
        for j in range(4):
            t[32 * j: 32 * j + ENC] = wmat[:, m * 128:(m + 1) * 128] * PS
            t[32 * j + ENC] = bvec[m * 128:(m + 1) * 128] * PS
        return t.astype(NPBF)

    for m in range(2):
        w[f"w0x{m}"] = blk4(inp["w0"], inp["b0"], m)
        w[f"w4ex{m}"] = blk4(inp["w4"][H:H + ENC], inp["b4"], m)

    # fp8 DoubleRow weights, scaled by WS
    for i in (1, 2, 3, 5, 6):
        w[f"wq{i}"] = (kstack(inp[f"w{i}"]) * WS).astype(NPF8)
    w["w4h"] = (kstack(inp["w4"][0:H]) * WS).astype(NPF8)
    w["w7f"] = (kstack(inp["w7"][:, 1:129]) * WS).astype(NPF8)

    # ACT-folded biases: AS*b per half as per-partition columns
    for i in (1, 2, 3, 5, 6):
        w[f"b{i}sAS"] = np.ascontiguousarray(
            inp[f"b{i}"].reshape(2, 128).T * AS).astype(np.float32)  # [128,2]

    # k=1 bias-matmul rows (PS-scaled) for the VecE-evac'd halves: the same
    # bias row duplicated at all four 32-row groups so the bias MMs of 4
    # consecutive chunks pack concurrently (rows 32*(c%4)).
    def bias4(vec):
        t = np.zeros((128, 128), dtype=np.float32)
        for j in range(4):
            t[32 * j] = vec * PS
        return t.astype(NPBF)

    for i in (1, 2, 3, 5, 6):
        w[f"biasM1L{i}"] = bias4(inp[f"b{i}"][128:256])
    w["biasL7"] = bias4(inp["b7"][1:129])

    # tail: density one-hot col i (of 8), DR fp8, w7[:,0]*WS. Cols padded to
    # 32 so the c==0 start=True matmul covers (and clears has_written for)
    # the FULL 32-partition tail region each group -- the color matmuls
    # (start=False) then overwrite/accumulate correctly; without this the
    # color partitions 16-31 keep stale accumulation across bank reuse.
    wd = np.zeros((128, 8, 2, 32), dtype=np.float32)
    for i in range(8):
        pad = np.zeros((256, 32), dtype=np.float32)
        pad[:, i] = inp["w7"][:, 0] * WS
        wd[:, i] = kstack(pad)
    w["w7d8"] = wd.astype(NPF8)
    # tail: color one-hot col 8 + 8c + i (c-plane-major for contiguous
    # gathers), bf16, unscaled (F1 carries the 32x)
    wc = np.zeros((128, 8, 32), dtype=np.float32)
    for i in range(8):
        for cc in range(3):
            wc[:, i, 8 + 8 * cc + i] = inp["w8"][0:128, cc]
    w["w8f8"] = wc.astype(NPBF)

    w["w8v3s"] = (inp["w8"][128:131] / WS).astype(np.float32)  # [3,3]
    w["b7d64"] = np.full((S, 1), inp["b7"][0], dtype=np.float32)
    for cc in range(3):
        w[f"b8c64_{cc}"] = np.full((S, 1), inp["b8"][cc], dtype=np.float32)
    return w


CONST_SPECS = {
    "w0x0": ((128, 128), BF16), "w0x1": ((128, 128), BF16),
    "w4ex0": ((128, 128), BF16), "w4ex1": ((128, 128), BF16),
    "wq1": ((128, 2, 256), FP8), "wq2": ((128, 2, 256), FP8),
    "wq3": ((128, 2, 256), FP8), "w4h": ((128, 2, 256), FP8),
    "wq5": ((128, 2, 256), FP8), "wq6": ((128, 2, 256), FP8),
    "w7f": ((128, 2, 128), FP8),
    "b1sAS": ((128, 2), F32), "b2sAS": ((128, 2), F32),
    "b3sAS": ((128, 2), F32), "b5sAS": ((128, 2), F32),
    "b6sAS": ((128, 2), F32),
    "biasM1L1": ((128, 128), BF16), "biasM1L2": ((128, 128), BF16),
    "biasM1L3": ((128, 128), BF16), "biasM1L5": ((128, 128), BF16),
    "biasM1L6": ((128, 128), BF16), "biasL7": ((128, 128), BF16),
    "w7d8": ((128, 8, 2, 32), FP8),
    "w8f8": ((128, 8, 32), BF16),
    "w8v3s": ((3, 3), F32),
    "b7d64": ((S, 1), F32),
    "b8c64_0": ((S, 1), F32), "b8c64_1": ((S, 1), F32), "b8c64_2": ((S, 1), F32),
    "fturn30": ((ENC, 1), F32), "phase30": ((ENC, 1), F32),
    "svec64": ((S, 1), F32), "srow64": ((S, 1), F32),
    "cap1e10": ((1, BL), F32), "q025": ((1, BL), F32),
    "ltri": ((S, S), BF16), "ltri2": ((S, S), BF16),
    "ones31": ((3, 1), F32), "ones641": ((S, 1), BF16),
}

IN_SPECS = {"xT": ((6, BL), F32), "off": ((S, BL), F32)}


def bcast_rows(ap, reps, cols):
    rows = ap.shape[0]
    return bass.AP(
        tensor=ap.tensor,
        offset=ap.offset,
        ap=[[ap.ap[0][0], rows], [0, reps], [1, cols]],
    )


def build_nerf(tc, ctx, out_ap, a, taps=None):
    nc = tc.nc
    B = BL

    consts = ctx.enter_context(tc.tile_pool(name="consts", bufs=1))
    pre = ctx.enter_context(tc.tile_pool(name="pre", bufs=1))
    work = ctx.enter_context(tc.tile_pool(name="work", bufs=2))
    psum = ctx.enter_context(tc.tile_pool(name="psum", bufs=1, space="PSUM"))

    # ---- constants / weights straight into SBUF (host pre-cast) ----
    # Small early constants (angle path, pre-phase, L0) are DMA'd first;
    # the bulky hidden-layer weights are deferred until after the pre-phase
    # and the first angle tiles are emitted, so they don't sit in front of
    # the latency-critical startup DMAs in the queue.
    EARLY = ("fturn30", "phase30", "srow64", "svec64", "cap1e10", "q025",
             "ones31", "w8v3s", "w0x0", "w0x1", "b7d64",
             "b8c64_0", "b8c64_1", "b8c64_2", "ltri", "ltri2", "ones641")
    sb = {}

    def load_consts(names):
        for name in names:
            shape, dt = CONST_SPECS[name]
            t = consts.tile(list(shape), dt, name=name, tag=name)
            nc.sync.dma_start(out=t, in_=a[name])
            sb[name] = t

    load_consts(EARLY)
    ones4 = consts.tile([128, B], BF16, name="ones4", tag="ones4")
    nc.vector.memset(ones4, 1.0)

    # ---- pre-phase ----
    dt3 = pre.tile([3, B], F32, name="dt3", tag="dt3")
    nc.sync.dma_start(out=dt3, in_=a["xT"][3:6])
    off = pre.tile([S, B], F32, name="off", tag="off")
    nc.sync.dma_start(out=off, in_=a["off"])

    D30 = pre.tile([ENC, B], F32, name="D30", tag="D30")
    nc.sync.dma_start(out=D30, in_=bcast_rows(a["xT"][3:6], 2 * L_ENC, B))
    O30 = pre.tile([ENC, B], F32, name="O30", tag="O30")
    nc.sync.dma_start(out=O30, in_=bcast_rows(a["xT"][0:3], 2 * L_ENC, B))
    DF = pre.tile([ENC, B], F32, name="DF", tag="DF")
    nc.vector.tensor_scalar(out=DF, in0=D30, scalar1=sb["fturn30"],
                            scalar2=None, op0=OP.mult)
    AO = pre.tile([ENC, B], F32, name="AO", tag="AO")
    nc.vector.tensor_scalar(out=AO, in0=O30, scalar1=sb["fturn30"],
                            scalar2=sb["phase30"], op0=OP.mult, op1=OP.add)
    DStep30 = pre.tile([ENC, B], F32, name="DStep30", tag="DStep30")
    nc.vector.tensor_scalar(out=DStep30, in0=DF, scalar1=float(DELTA),
                            scalar2=None, op0=OP.mult)
    Qb30 = pre.tile([ENC, B], F32, name="Qb30", tag="Qb30")
    nc.vector.affine_then_add(out=Qb30, in0=DF, in1=AO, scale=float(NEAR),
                              bias=0.0)

    # assemble 4-block [128,B] versions; rows 30 of each block: DStep=0,
    # Qb=0.25 (bias row -> sin=1); rows 31: 0 (zero pad).
    DStep4 = pre.tile([128, B], F32, name="DStep4", tag="DStep4")
    nc.vector.memset(DStep4, 0.0)
    Qb4 = pre.tile([128, B], F32, name="Qb4", tag="Qb4")
    nc.vector.memset(Qb4, 0.0)
    for j in range(4):
        nc.sync.dma_start(out=DStep4[32 * j: 32 * j + ENC], in_=DStep30)
        nc.sync.dma_start(out=Qb4[32 * j: 32 * j + ENC], in_=Qb30)
        nc.sync.dma_start(out=Qb4[32 * j + ENC: 32 * j + ENC + 1],
                          in_=a["q025"])

    off_plus = pre.tile([S, B], F32, name="off_plus", tag="off_plus")
    nc.vector.tensor_scalar(out=off_plus, in0=off, scalar1=sb["srow64"],
                            scalar2=None, op0=OP.add)

    # |d| and view-dir color contribution
    sq3 = pre.tile([3, B], F32, name="sq3", tag="sq3")
    nc.vector.tensor_mul(sq3, dt3, dt3)
    p0 = psum.tile([128, 2, 512], F32, name="pp", tag="pp", bufs=4)
    nc.tensor.matmul(p0[0:1, 0, :B], sb["ones31"], sq3, start=True, stop=True)
    nd = pre.tile([1, B], F32, name="nd", tag="nd")
    nc.scalar.activation(out=nd, in_=p0[0:1, 0, :B], func=AF.Sqrt)
    inv_nd = pre.tile([1, B], F32, name="inv_nd", tag="inv_nd")
    nc.vector.reciprocal(out=inv_nd, in_=nd)
    inv3 = pre.tile([3, B], F32, name="inv3", tag="inv3")
    nc.gpsimd.partition_broadcast(inv3, inv_nd)
    v3 = pre.tile([3, B], F32, name="v3", tag="v3")
    nc.vector.tensor_mul(v3, dt3, inv3)
    p1 = psum.tile([128, 2, 512], F32, name="pp", tag="pp", bufs=4)
    nc.tensor.matmul(p1[0:3, 0, :B], sb["w8v3s"], v3, start=True, stop=True)
    vc3 = pre.tile([3, B], F32, name="vc3", tag="vc3")
    nc.scalar.activation(out=vc3, in_=p1[0:3, 0, :B], func=AF.Copy)
    vcb = []
    for cc in range(3):
        t = pre.tile([S, B], F32, name=f"vcb{cc}", tag=f"vcb{cc}")
        nc.sync.dma_start(out=t, in_=bcast_rows(vc3[cc:cc + 1], S, B))
        vcb.append(t)

    # dists
    Z = pre.tile([S, B], F32, name="Z", tag="Z")
    nc.vector.tensor_scalar(out=Z, in0=off, scalar1=float(DELTA),
                            scalar2=sb["svec64"], op0=OP.mult, op1=OP.add)
    nd64 = pre.tile([S, B], F32, name="nd64", tag="nd64")
    nc.gpsimd.partition_broadcast(nd64, nd)
    ZN = pre.tile([S, B], F32, name="ZN", tag="ZN")
    nc.vector.tensor_mul(ZN, Z, nd64)
    ZNs = pre.tile([S, B], F32, name="ZNs", tag="ZNs")
    nc.sync.dma_start(out=ZNs[0: S - 1], in_=ZN[1:S])
    nc.sync.dma_start(out=ZNs[S - 1: S], in_=a["cap1e10"])
    dists = pre.tile([S, B], F32, name="dists", tag="dists")
    nc.vector.tensor_sub(dists, ZNs, ZN)

    # CT destination slabs ([32,B] per 4-chunk half-group, 16 slabs)
    CT = [pre.tile([128, B], F32, name=f"CT{i}", tag=f"CT{i}")
          for i in range(4)]

    # ---- angle tiles (4 chunks each) ----
    enc_tiles = {}

    def emit_angle(ti):
        OFF4 = work.tile([128, B], F32, name=f"off4_{ti}", tag="off4", bufs=4)
        for j in range(4):
            s = 4 * ti + j
            nc.sync.dma_start(out=OFF4[32 * j: 32 * j + 32],
                              in_=bcast_rows(off_plus[s:s + 1], 32, B))
        um = work.tile([128, B], F32, name=f"um{ti}", tag="um", bufs=3)
        nc.gpsimd.tensor_mul(um, OFF4, DStep4)
        uu = work.tile([128, B], F32, name=f"uu{ti}", tag="uu", bufs=3)
        nc.gpsimd.tensor_add(uu, um, Qb4)
        kk = work.tile([128, B], F32, name=f"kk{ti}", tag="kk", bufs=3)
        nc.vector.tensor_scalar(out=kk, in0=uu, scalar1=MAGIC, scalar2=MAGIC,
                                op0=OP.add, op1=OP.subtract)
        ff = work.tile([128, B], F32, name=f"ff{ti}", tag="ff", bufs=3)
        nc.gpsimd.tensor_sub(ff, uu, kk)
        e = work.tile([128, B], BF16, name=f"enc{ti}", tag="enc", bufs=9)
        nc.scalar.activation(out=e, in_=ff, func=AF.Sin, scale=TWO_PI)
        enc_tiles[ti] = e

    emit_angle(0)
    emit_angle(1)
    load_consts([n for n in CONST_SPECS if n not in EARLY])

    # gather destinations, filled per-slab during the main loop
    D64 = pre.tile([S, B], F32, name="D64", tag="D64")
    THg = [pre.tile([S, B], F32, name=f"TH{cc}", tag=f"TH{cc}")
           for cc in range(3)]

    def emit_gathers(sidx):
        src = CT[sidx // 4]
        q = sidx % 4
        nc.sync.dma_start(out=D64[4 * sidx: 4 * sidx + 4],
                          in_=src[32 * q: 32 * q + 4])
        for cc in range(3):
            nc.sync.dma_start(
                out=THg[cc][4 * sidx: 4 * sidx + 4],
                in_=src[32 * q + 8 + 8 * cc: 32 * q + 12 + 8 * cc])

    # evac engine per (layer, half): ScalarE folds the bias (relu+scale+bias
    # in one ACTIVATE); VectorE halves take bias from PSUM ('mm': packed k=1
    # bias matmuls) or from the encoding rows ('enc': L0/L4 carry [w;b]).
    # Strict engine alternation: every layer's m0 half on ScalarE (folded
    # bias), m1 half on VectorE (bias from PSUM via packed k=1 matmuls, or
    # from the encoding rows for L0/L4). Consecutive psum-slot evacs then
    # ping-pong between the two PSUM-draining engines, keeping both busy.
    EVH = {}
    for _li in range(7):
        EVH[(_li, 0)] = ("act", "enc" if _li in (0, 4) else None)
        EVH[(_li, 1)] = ("vec", "enc" if _li in (0, 4) else "mm")
    BIAS_MM = {(li, 1): f"biasM1L{li}" for li in (1, 2, 3, 5, 6)}
    BIAS_MM[7] = "biasL7"

    def evac_half(li, m, pp_slot, hpair):
        # pp_slot [128, 2(chunk), 512] -> hpair[:, m, :, :]
        eng, bmode = EVH[(li, m)]
        out = hpair[:, m, :, :]
        if eng == "act":
            bias = 0.0 if bmode == "enc" else sb[f"b{li}sAS"][:, m:m + 1]
            nc.scalar.activation(out=out, in_=pp_slot[:, :, :B], func=AF.Relu,
                                 scale=INV_WS, bias=bias)
        else:
            nc.vector.tensor_scalar(out=out, in0=pp_slot[:, :, :B],
                                    scalar1=0.0, scalar2=INV_WS,
                                    op0=OP.max, op1=OP.mult)

    # ---- main loop: super-group PAIRS, two interleaved dependency chains
    # so the 4-slot psum window always has ready work from one stream while
    # the other waits on its evacs ----
    emit_angle(2)
    emit_angle(3)

    def enc_of_g(g):
        eA, eB = enc_tiles[2 * g], enc_tiles[2 * g + 1]

        def enc_of(c):
            return (eA if c < 4 else eB), 32 * (c % 4)
        return enc_of

    def new_hpair():
        return work.tile([128, 2, 2, B], FP8, name="hp", tag="hp", bufs=18)

    def slot():
        return psum.tile([128, 2, 512], F32, name="pp", tag="pp", bufs=4)

    # L0 / L4: k=31 enc matmuls (row-group packed); halves interleaved per
    # pair so consecutive slot evacs alternate engines
    def enc_layer(li, wname, h_in, enc_of):
        hp = [new_hpair() for _ in range(4)]
        for pr in range(4):
            for m in range(2):
                sl = slot()
                for ci in range(2):
                    c = 2 * pr + ci
                    e, rb = enc_of(c)
                    nc.tensor.matmul(sl[:, ci, :B],
                                     sb[f"{wname}{m}"][rb:rb + 32],
                                     e[rb:rb + 32], start=True,
                                     stop=(h_in is None),
                                     tile_position=(rb, 0))
                if h_in is not None:
                    for ci in range(2):
                        c = 2 * pr + ci
                        nc.tensor.matmul(
                            sl[:, ci, :B],
                            sb["w4h"][:, :, 128 * m:128 * m + 128],
                            h_in[c // 2][:, :, c % 2, :],
                            start=False, stop=True, perf_mode=DR)
                evac_half(li, m, sl, hp[pr])
        return hp

    # DR hidden layers: halves interleaved per pair; bias matmuls for the
    # 'mm' half batched 4-packed at the head of each 4-chunk sub-phase
    def dr_layer(li, h_in):
        hp = [new_hpair() for _ in range(4)]
        has_bias = (li, 1) in BIAS_MM
        for hg in range(2):
            bslots = {}
            if has_bias:
                bl = sb[BIAS_MM[(li, 1)]]
                for p in (2 * hg, 2 * hg + 1):
                    bslots[p] = slot()
                for c4 in range(4):
                    c = 4 * hg + c4
                    r = 32 * (c % 4)
                    nc.tensor.matmul(bslots[c // 2][:, c % 2, :B],
                                     bl[r:r + 1], ones4[r:r + 1],
                                     start=True, stop=False,
                                     tile_position=(r, 0))
            for p in (2 * hg, 2 * hg + 1):
                for m in range(2):
                    sl = bslots[p] if (has_bias and m == 1) else slot()
                    for ci in range(2):
                        c = 2 * p + ci
                        nc.tensor.matmul(
                            sl[:, ci, :B],
                            sb[f"wq{li}"][:, :, 128 * m:128 * m + 128],
                            h_in[c // 2][:, :, c % 2, :],
                            start=(not (has_bias and m == 1)), stop=True,
                            perf_mode=DR)
                    evac_half(li, m, sl, hp[p])
        return hp

    def l7f_tail(g, h_prev):
        # L7f: bias MMs batched per 4 chunks, then DR run; alternating evac
        F1p = [work.tile([128, 2, B], BF16, name="F1p", tag="F1p", bufs=8)
               for _ in range(4)]
        bl = sb["biasL7"]
        for hg in range(2):
            slots = [slot(), slot()]
            for c4 in range(4):
                c = 4 * hg + c4
                r = 32 * (c % 4)
                nc.tensor.matmul(slots[c4 // 2][:, c % 2, :B],
                                 bl[r:r + 1], ones4[r:r + 1],
                                 start=True, stop=False, tile_position=(r, 0))
            for c4 in range(4):
                c = 4 * hg + c4
                nc.tensor.matmul(slots[c4 // 2][:, c % 2, :B], sb["w7f"],
                                 h_prev[c // 2][:, :, c % 2, :],
                                 start=False, stop=True, perf_mode=DR)
            for pr in range(2):
                p_idx = 2 * hg + pr
                if p_idx % 2 == 0:
                    nc.scalar.activation(out=F1p[p_idx],
                                         in_=slots[pr][:, :, :B],
                                         func=AF.Relu, scale=INV_WS)
                else:
                    nc.vector.tensor_scalar(out=F1p[p_idx],
                                            in0=slots[pr][:, :, :B],
                                            scalar1=0.0, scalar2=INV_WS,
                                            op0=OP.max, op1=OP.mult)
        # tail: density + colors accumulate per 4-chunk half-group in a
        # pool slot (bank 0)
        for hg in range(2):
            T = slot()
            for c4 in range(4):
                c = 4 * hg + c4
                nc.tensor.matmul(T[0:32, 0, :B], sb["w7d8"][:, c4, :, :],
                                 h_prev[c // 2][:, :, c % 2, :],
                                 start=(c4 == 0), stop=False, perf_mode=DR,
                                 skip_group_check=True)
            for c4 in range(4):
                c = 4 * hg + c4
                nc.tensor.matmul(T[0:32, 0, :B], sb["w8f8"][:, c4, :],
                                 F1p[c // 2][:, c % 2, :],
                                 start=False, stop=(c4 == 3),
                                 skip_group_check=True)
            sidx = 2 * g + hg
            tgt = CT[sidx // 4]
            q = sidx % 4
            nc.scalar.activation(out=tgt[32 * q: 32 * q + 32],
                                 in_=T[0:32, 0, :B], func=AF.Copy,
                                 scale=INV_PS)
            emit_gathers(sidx)

    for gp in range(4):
        if gp < 3:
            for t in range(4 * gp + 4, 4 * gp + 8):
                emit_angle(t)
        eofs = [enc_of_g(2 * gp), enc_of_g(2 * gp + 1)]
        hs = [enc_layer(0, "w0x", None, eofs[dg]) for dg in range(2)]
        if taps is not None and gp == 0:
            taps["enc0"] = enc_tiles[0]
            taps["h0"] = hs[0][0]
        for li in (1, 2, 3):
            for dg in range(2):
                hs[dg] = dr_layer(li, hs[dg])
        for dg in range(2):
            hs[dg] = enc_layer(4, "w4ex", hs[dg], eofs[dg])
        for li in (5, 6):
            for dg in range(2):
                hs[dg] = dr_layer(li, hs[dg])
        if taps is not None and gp == 0:
            taps["h6"] = hs[0][0]
        for dg in range(2):
            l7f_tail(2 * gp + dg, hs[dg])

    # ---- compositing (gathers already streamed during the main loop) ----
    SG = pre.tile([S, B], F32, name="SG", tag="SG")
    nc.scalar.activation(out=SG, in_=D64, func=AF.Relu, bias=sb["b7d64"],
                         scale=1.0)
    M64 = pre.tile([S, B], BF16, name="M64", tag="M64")
    nc.vector.tensor_mul(M64, SG, dists)
    pc2 = psum.tile([128, 2, 512], F32, name="pp", tag="pp", bufs=4)
    nc.tensor.matmul(pc2[0:S, 0, :B], sb["ltri"], M64, start=True, stop=True)
    nc.tensor.matmul(pc2[0:S, 1, :B], sb["ltri2"], M64, start=True, stop=True)
    T64 = pre.tile([S, B], F32, name="T64", tag="T64")
    nc.scalar.activation(out=T64, in_=pc2[0:S, 0, :B], func=AF.Exp, scale=-1.0)
    T64b = pre.tile([S, B], F32, name="T64b", tag="T64b")
    nc.scalar.activation(out=T64b, in_=pc2[0:S, 1, :B], func=AF.Exp, scale=-1.0)
    w64 = pre.tile([S, B], BF16, name="w64", tag="w64")
    nc.vector.tensor_sub(w64, T64, T64b)
    if taps is not None:
        taps["D64"] = D64
        taps["w64"] = w64
        taps["CTa"] = CT[0]
        taps["CTb"] = CT[1]

    for cc in range(3):
        Sc = pre.tile([S, B], F32, name=f"Sc{cc}", tag=f"Sc{cc}")
        nc.vector.tensor_add(Sc, THg[cc], vcb[cc])
        St = pre.tile([S, B], BF16, name=f"St{cc}", tag=f"St{cc}")
        nc.scalar.activation(out=St, in_=Sc, func=AF.Sigmoid, scale=WS,
                             bias=sb[f"b8c64_{cc}"])
        P = pre.tile([S, B], BF16, name=f"P{cc}", tag=f"P{cc}")
        nc.vector.tensor_mul(P, w64, St)
        pc = psum.tile([128, 2, 512], F32, name="pp", tag="pp", bufs=4)
        nc.tensor.matmul(pc[0:1, 0, :B], sb["ones641"], P, start=True, stop=True)
        oc = pre.tile([1, B], F32, name=f"oc{cc}", tag=f"oc{cc}")
        nc.scalar.activation(out=oc, in_=pc[0:1, 0, :B], func=AF.Copy)
        nc.sync.dma_start(out=out_ap.rearrange("b c -> c b")[cc:cc + 1], in_=oc)

    if taps is not None:
        for tname, t in taps.items():
            nc.sync.dma_start(out=a["tap_" + tname], in_=t)


TAP_SPECS = {
    "enc0": ([128, BL], BF16), "h0": ([128, 2, 2, BL], FP8),
    "D64": ([S, BL], F32), "w64": ([S, BL], BF16), "CTa": ([128, BL], F32),
    "CTb": ([128, BL], F32), "h6": ([128, 2, 2, BL], FP8),
}


def build_nc(debug_taps=False):
    nc = bacc.Bacc("TRN2", target_bir_lowering=False, debug=False)
    aps = {}
    for name, (shape, dt) in {**IN_SPECS, **CONST_SPECS}.items():
        aps[name] = nc.dram_tensor(name, list(shape), dt,
                                   kind="ExternalInput").ap()
    taps = None
    if debug_taps:
        taps = {}
        for tname, (shape, dt) in TAP_SPECS.items():
            aps["tap_" + tname] = nc.dram_tensor(
                "tap_" + tname, shape, dt, kind="ExternalOutput").ap()
    out = nc.dram_tensor("out", [BL, 3], F32, kind="ExternalOutput").ap()
    with tile.TileContext(nc) as tc, ExitStack() as ctx:
        build_nerf(tc, ctx, out, aps, taps=taps)
    nc.compile()
    return nc


def make_in_maps(inputs):
    consts = host_constants()
    wts = host_weights(inputs)
    shared = {**consts, **wts}
    for name, (shape, dt) in CONST_SPECS.items():
        v = np.ascontiguousarray(shared[name])
        assert v.shape == tuple(shape), (name, v.shape, shape)
        shared[name] = v
    in_maps = []
    for core in range(N_CORES):
        sl = slice(core * BL, (core + 1) * BL)
        m = dict(shared)
        m["xT"] = np.ascontiguousarray(np.asarray(inputs["x"])[sl].T,
                                       dtype=np.float32)
        m["off"] = np.ascontiguousarray(
            np.asarray(inputs["offsets"])[:, sl], dtype=np.float32)
        in_maps.append(m)
    return in_maps


def kernel(**inputs):
    from concourse.bass_utils import run_bass_kernel_spmd

    nc = build_nc()
    in_maps = make_in_maps(inputs)
    res = run_bass_kernel_spmd(nc, in_maps, core_ids=list(range(N_CORES)))
    out = np.concatenate([r["out"] for r in res.results], axis=0)
    return out.astype(np.float32)


# revision 66
# speedup vs baseline: 1.0226x; 1.0226x over previous
"""NeRF MLP kernel for Trainium2 (Bass/Tile), 8-core data-parallel over rays.

v3 design — layer-major, weight-stationary, evac-balanced:

- Layout: features on SBUF partitions, rays (512/core) on the free dim; one
  "chunk" = one sample index s for all local rays.  Chunks are processed in
  super-groups of 8, LAYER-major within the group, so each layer's weights
  stay stationary in the PE across 8 back-to-back matmuls (keeps the PE
  dense/warm; the HAM clock-gate stays at 8/8).
- Hidden matmuls are fp8e4 DoubleRow (weights host-scaled by 512,
  activations stored as 32*h): each 256-contraction layer half is one PE
  matmul at N=512.
- ALL biases are pre-accumulated into PSUM: L0/L4 carry [w;b] rows in the
  k=31 encoding matmuls (enc row 30 is sin(pi/2)=1); L1-L7f get packed k=1
  bias matmuls (rows at 32-aligned groups, even/odd chunks use different
  groups so adjacent bias matmuls pack concurrently).  Every PSUM->SBUF
  evacuation is then a single instruction: relu+scale on ScalarE or
  max0+mult on VectorE — the two PSUM-capable engines, which are the
  throughput floor of this kernel (~1 elem/cycle/partition each).
- Encoding uses the EXACT per-sample z (offsets do not cancel): angle
  u = (s+off[s])*DStep + Qb computed per 4-chunk tile on GpSimd (idle
  otherwise), magic-add range reduction on VectorE, one ScalarE Sin.
- Density (L7d) and color (w8f) rows accumulate IN PSUM across each
  super-group via one-hot weight columns — one cheap [32,512] evac per 8
  chunks instead of per-chunk copies.
- Compositing: w_s = exp(-cumsum) difference via bf16 triangular matmuls.
"""

import math
from contextlib import ExitStack

import numpy as np
import ml_dtypes

import concourse.bass as bass
import concourse.mybir as mybir
import concourse.tile as tile
from concourse import bacc

F32 = mybir.dt.float32
BF16 = mybir.dt.bfloat16
FP8 = mybir.dt.float8e4
AF = mybir.ActivationFunctionType
OP = mybir.AluOpType
DR = mybir.MatmulPerfMode.DoubleRow

S = 64          # samples per ray
B_FULL = 4096   # total rays
N_CORES = 8
BL = B_FULL // N_CORES  # rays per core = 512
H = 256
NEAR, FAR = 2.0, 6.0
DELTA = (FAR - NEAR) / S
L_ENC = 5
ENC = 3 * L_ENC * 2  # 30
TWO_PI = 2.0 * math.pi
MAGIC = 12582912.0  # 1.5 * 2**23, fp32 round-to-nearest trick

WS = 512.0      # fp8 weight scale
AS = 32.0       # fp8 activation scale (stored act = 32*h)
PS = WS * AS    # psum scale for biased layers = 16384
INV_WS = 1.0 / WS    # 2^-9: psum -> stored-act scale
INV_PS = 1.0 / PS    # 2^-14: tail evac scale

NPF8 = ml_dtypes.float8_e4m3
NPBF = ml_dtypes.bfloat16

# evac engine per hidden layer (7 = L7f). 'vec' layers use max0+mult
# tensor_scalar; 'act' layers use Relu activation. All read bias from PSUM.
EV = {0: "vec", 1: "vec", 2: "act", 3: "act", 4: "vec", 5: "act", 6: "act",
      7: "vec"}


def host_constants():
    c = {}
    freqs = (2.0 ** (np.arange(L_ENC, dtype=np.float64) - 2)) * math.pi  # [L]
    fturn = np.zeros((ENC, 1), dtype=np.float32)
    phase = np.zeros((ENC, 1), dtype=np.float32)
    for cc in range(3):
        for ll in range(L_ENC):
            for tt in range(2):
                j = cc * (L_ENC * 2) + ll * 2 + tt
                fturn[j, 0] = freqs[ll] / TWO_PI
                phase[j, 0] = 0.0 if tt == 0 else 0.25  # pi/2 in turns
    c["fturn30"] = fturn
    c["phase30"] = phase
    c["cap1e10"] = np.full((1, BL), 1.0e10, dtype=np.float32)
    c["q025"] = np.full((1, BL), 0.25, dtype=np.float32)
    c["svec64"] = (NEAR + np.arange(S, dtype=np.float32)[:, None] * DELTA)
    c["srow64"] = np.arange(S, dtype=np.float32)[:, None]
    c["ltri"] = np.triu(np.ones((S, S), dtype=np.float32)).astype(NPBF)
    c["ltri2"] = (np.triu(np.ones((S, S))) + np.eye(S)).astype(NPBF)
    c["ones31"] = np.ones((3, 1), dtype=np.float32)
    c["ones641"] = np.ones((S, 1), dtype=NPBF)
    return c


def host_weights(inp):
    w = {}

    def kstack(m):  # [256, M] -> [128, 2, M]
        return np.ascontiguousarray(m.reshape(2, 128, m.shape[1]).transpose(1, 0, 2))

    # k=31 encoding matmuls carry weight+bias scaled by PS: 4 identical
    # 32-row blocks (one per chunk slot in a 4-chunk enc tile); row 30 is
    # the bias row (enc row 30 evaluates to 1), row 31 zero.
    def blk4(wmat, bvec, m):
        t = np.zeros((128, 128), dtype=np.float32)
        for j in range(4):
            t[32 * j: 32 * j + ENC] = wmat[:, m * 128:(m + 1) * 128] * PS
            t[32 * j + ENC] = bvec[m * 128:(m + 1) * 128] * PS
        return t.astype(NPBF)

    for m in range(2):
        w[f"w0x{m}"] = blk4(inp["w0"], inp["b0"], m)
        w[f"w4ex{m}"] = blk4(inp["w4"][H:H + ENC], inp["b4"], m)

    # fp8 DoubleRow weights, scaled by WS
    for i in (1, 2, 3, 5, 6):
        w[f"wq{i}"] = (kstack(inp[f"w{i}"]) * WS).astype(NPF8)
    w["w4h"] = (kstack(inp["w4"][0:H]) * WS).astype(NPF8)
    w["w7f"] = (kstack(inp["w7"][:, 1:129]) * WS).astype(NPF8)

    # ACT-folded biases: AS*b per half as per-partition columns
    for i in (1, 2, 3, 5, 6):
        w[f"b{i}sAS"] = np.ascontiguousarray(
            inp[f"b{i}"].reshape(2, 128).T * AS).astype(np.float32)  # [128,2]

    # k=1 bias-matmul rows (PS-scaled) for the VecE-evac'd halves: the same
    # bias row duplicated at all four 32-row groups so the bias MMs of 4
    # consecutive chunks pack concurrently (rows 32*(c%4)).
    def bias4(vec):
        t = np.zeros((128, 128), dtype=np.float32)
        for j in range(4):
            t[32 * j] = vec * PS
        return t.astype(NPBF)

    for i in (1, 2, 3, 5, 6):
        w[f"biasM1L{i}"] = bias4(inp[f"b{i}"][128:256])
    w["biasL7"] = bias4(inp["b7"][1:129])

    # tail: density one-hot col i (of 8), DR fp8, w7[:,0]*WS. Cols padded to
    # 32 so the c==0 start=True matmul covers (and clears has_written for)
    # the FULL 32-partition tail region each group -- the color matmuls
    # (start=False) then overwrite/accumulate correctly; without this the
    # color partitions 16-31 keep stale accumulation across bank reuse.
    wd = np.zeros((128, 8, 2, 32), dtype=np.float32)
    for i in range(8):
        pad = np.zeros((256, 32), dtype=np.float32)
        pad[:, i] = inp["w7"][:, 0] * WS
        wd[:, i] = kstack(pad)
    w["w7d8"] = wd.astype(NPF8)
    # tail: color one-hot col 8 + 8c + i (c-plane-major for contiguous
    # gathers), bf16, unscaled (F1 carries the 32x)
    wc = np.zeros((128, 8, 32), dtype=np.float32)
    for i in range(8):
        for cc in range(3):
            wc[:, i, 8 + 8 * cc + i] = inp["w8"][0:128, cc]
    w["w8f8"] = wc.astype(NPBF)

    w["w8v3s"] = (inp["w8"][128:131] / WS).astype(np.float32)  # [3,3]
    w["b7d64"] = np.full((S, 1), inp["b7"][0], dtype=np.float32)
    for cc in range(3):
        w[f"b8c64_{cc}"] = np.full((S, 1), inp["b8"][cc], dtype=np.float32)
    return w


CONST_SPECS = {
    "w0x0": ((128, 128), BF16), "w0x1": ((128, 128), BF16),
    "w4ex0": ((128, 128), BF16), "w4ex1": ((128, 128), BF16),
    "wq1": ((128, 2, 256), FP8), "wq2": ((128, 2, 256), FP8),
    "wq3": ((128, 2, 256), FP8), "w4h": ((128, 2, 256), FP8),
    "wq5": ((128, 2, 256), FP8), "wq6": ((128, 2, 256), FP8),
    "w7f": ((128, 2, 128), FP8),
    "b1sAS": ((128, 2), F32), "b2sAS": ((128, 2), F32),
    "b3sAS": ((128, 2), F32), "b5sAS": ((128, 2), F32),
    "b6sAS": ((128, 2), F32),
    "biasM1L1": ((128, 128), BF16), "biasM1L2": ((128, 128), BF16),
    "biasM1L3": ((128, 128), BF16), "biasM1L5": ((128, 128), BF16),
    "biasM1L6": ((128, 128), BF16), "biasL7": ((128, 128), BF16),
    "w7d8": ((128, 8, 2, 32), FP8),
    "w8f8": ((128, 8, 32), BF16),
    "w8v3s": ((3, 3), F32),
    "b7d64": ((S, 1), F32),
    "b8c64_0": ((S, 1), F32), "b8c64_1": ((S, 1), F32), "b8c64_2": ((S, 1), F32),
    "fturn30": ((ENC, 1), F32), "phase30": ((ENC, 1), F32),
    "svec64": ((S, 1), F32), "srow64": ((S, 1), F32),
    "cap1e10": ((1, BL), F32), "q025": ((1, BL), F32),
    "ltri": ((S, S), BF16), "ltri2": ((S, S), BF16),
    "ones31": ((3, 1), F32), "ones641": ((S, 1), BF16),
}

IN_SPECS = {"xT": ((6, BL), F32), "off": ((S, BL), F32)}


def bcast_rows(ap, reps, cols):
    rows = ap.shape[0]
    return bass.AP(
        tensor=ap.tensor,
        offset=ap.offset,
        ap=[[ap.ap[0][0], rows], [0, reps], [1, cols]],
    )


def build_nerf(tc, ctx, out_ap, a, taps=None):
    nc = tc.nc
    B = BL

    consts = ctx.enter_context(tc.tile_pool(name="consts", bufs=1))
    pre = ctx.enter_context(tc.tile_pool(name="pre", bufs=1))
    work = ctx.enter_context(tc.tile_pool(name="work", bufs=2))
    psum = ctx.enter_context(tc.tile_pool(name="psum", bufs=1, space="PSUM"))

    # ---- constants / weights straight into SBUF (host pre-cast) ----
    # Small early constants (angle path, pre-phase, L0) are DMA'd first;
    # the bulky hidden-layer weights are deferred until after the pre-phase
    # and the first angle tiles are emitted, so they don't sit in front of
    # the latency-critical startup DMAs in the queue.
    EARLY = ("fturn30", "phase30", "srow64", "svec64", "cap1e10", "q025",
             "ones31", "w8v3s", "w0x0", "w0x1", "b7d64",
             "b8c64_0", "b8c64_1", "b8c64_2", "ltri", "ltri2", "ones641")
    sb = {}

    def load_consts(names):
        for name in names:
            shape, dt = CONST_SPECS[name]
            t = consts.tile(list(shape), dt, name=name, tag=name)
            nc.sync.dma_start(out=t, in_=a[name])
            sb[name] = t

    load_consts(EARLY)
    ones4 = consts.tile([128, B], BF16, name="ones4", tag="ones4")
    nc.vector.memset(ones4, 1.0)

    # ---- pre-phase ----
    dt3 = pre.tile([3, B], F32, name="dt3", tag="dt3")
    nc.sync.dma_start(out=dt3, in_=a["xT"][3:6])
    off = pre.tile([S, B], F32, name="off", tag="off")
    nc.sync.dma_start(out=off, in_=a["off"])

    D30 = pre.tile([ENC, B], F32, name="D30", tag="D30")
    nc.sync.dma_start(out=D30, in_=bcast_rows(a["xT"][3:6], 2 * L_ENC, B))
    O30 = pre.tile([ENC, B], F32, name="O30", tag="O30")
    nc.sync.dma_start(out=O30, in_=bcast_rows(a["xT"][0:3], 2 * L_ENC, B))
    DF = pre.tile([ENC, B], F32, name="DF", tag="DF")
    nc.vector.tensor_scalar(out=DF, in0=D30, scalar1=sb["fturn30"],
                            scalar2=None, op0=OP.mult)
    AO = pre.tile([ENC, B], F32, name="AO", tag="AO")
    nc.vector.tensor_scalar(out=AO, in0=O30, scalar1=sb["fturn30"],
                            scalar2=sb["phase30"], op0=OP.mult, op1=OP.add)
    DStep30 = pre.tile([ENC, B], F32, name="DStep30", tag="DStep30")
    nc.vector.tensor_scalar(out=DStep30, in0=DF, scalar1=float(DELTA),
                            scalar2=None, op0=OP.mult)
    Qb30 = pre.tile([ENC, B], F32, name="Qb30", tag="Qb30")
    nc.vector.affine_then_add(out=Qb30, in0=DF, in1=AO, scale=float(NEAR),
                              bias=0.0)

    # assemble 4-block [128,B] versions; rows 30 of each block: DStep=0,
    # Qb=0.25 (bias row -> sin=1); rows 31: 0 (zero pad).
    DStep4 = pre.tile([128, B], F32, name="DStep4", tag="DStep4")
    nc.vector.memset(DStep4, 0.0)
    Qb4 = pre.tile([128, B], F32, name="Qb4", tag="Qb4")
    nc.vector.memset(Qb4, 0.0)
    for j in range(4):
        nc.sync.dma_start(out=DStep4[32 * j: 32 * j + ENC], in_=DStep30)
        nc.sync.dma_start(out=Qb4[32 * j: 32 * j + ENC], in_=Qb30)
        nc.sync.dma_start(out=Qb4[32 * j + ENC: 32 * j + ENC + 1],
                          in_=a["q025"])

    off_plus = pre.tile([S, B], F32, name="off_plus", tag="off_plus")
    nc.vector.tensor_scalar(out=off_plus, in0=off, scalar1=sb["srow64"],
                            scalar2=None, op0=OP.add)

    # |d| and view-dir color contribution
    sq3 = pre.tile([3, B], F32, name="sq3", tag="sq3")
    nc.vector.tensor_mul(sq3, dt3, dt3)
    p0 = psum.tile([128, 2, 512], F32, name="pp", tag="pp", bufs=4)
    nc.tensor.matmul(p0[0:1, 0, :B], sb["ones31"], sq3, start=True, stop=True)
    nd = pre.tile([1, B], F32, name="nd", tag="nd")
    nc.scalar.activation(out=nd, in_=p0[0:1, 0, :B], func=AF.Sqrt)
    inv_nd = pre.tile([1, B], F32, name="inv_nd", tag="inv_nd")
    nc.vector.reciprocal(out=inv_nd, in_=nd)
    inv3 = pre.tile([3, B], F32, name="inv3", tag="inv3")
    nc.gpsimd.partition_broadcast(inv3, inv_nd)
    v3 = pre.tile([3, B], F32, name="v3", tag="v3")
    nc.vector.tensor_mul(v3, dt3, inv3)
    p1 = psum.tile([128, 2, 512], F32, name="pp", tag="pp", bufs=4)
    nc.tensor.matmul(p1[0:3, 0, :B], sb["w8v3s"], v3, start=True, stop=True)
    vc3 = pre.tile([3, B], F32, name="vc3", tag="vc3")
    nc.scalar.activation(out=vc3, in_=p1[0:3, 0, :B], func=AF.Copy)
    vcb = []
    for cc in range(3):
        t = pre.tile([S, B], F32, name=f"vcb{cc}", tag=f"vcb{cc}")
        nc.sync.dma_start(out=t, in_=bcast_rows(vc3[cc:cc + 1], S, B))
        vcb.append(t)

    # dists
    Z = pre.tile([S, B], F32, name="Z", tag="Z")
    nc.vector.tensor_scalar(out=Z, in0=off, scalar1=float(DELTA),
                            scalar2=sb["svec64"], op0=OP.mult, op1=OP.add)
    nd64 = pre.tile([S, B], F32, name="nd64", tag="nd64")
    nc.gpsimd.partition_broadcast(nd64, nd)
    ZN = pre.tile([S, B], F32, name="ZN", tag="ZN")
    nc.vector.tensor_mul(ZN, Z, nd64)
    ZNs = pre.tile([S, B], F32, name="ZNs", tag="ZNs")
    nc.sync.dma_start(out=ZNs[0: S - 1], in_=ZN[1:S])
    nc.sync.dma_start(out=ZNs[S - 1: S], in_=a["cap1e10"])
    dists = pre.tile([S, B], F32, name="dists", tag="dists")
    nc.vector.tensor_sub(dists, ZNs, ZN)

    # CT destination slabs ([32,B] per 4-chunk half-group, 16 slabs)
    CT = [pre.tile([128, B], F32, name=f"CT{i}", tag=f"CT{i}")
          for i in range(4)]

    # ---- angle tiles (4 chunks each) ----
    enc_tiles = {}

    def emit_angle(ti):
        OFF4 = work.tile([128, B], F32, name=f"off4_{ti}", tag="off4", bufs=4)
        for j in range(4):
            s = 4 * ti + j
            nc.sync.dma_start(out=OFF4[32 * j: 32 * j + 32],
                              in_=bcast_rows(off_plus[s:s + 1], 32, B))
        um = work.tile([128, B], F32, name=f"um{ti}", tag="um", bufs=3)
        nc.gpsimd.tensor_mul(um, OFF4, DStep4)
        uu = work.tile([128, B], F32, name=f"uu{ti}", tag="uu", bufs=3)
        nc.gpsimd.tensor_add(uu, um, Qb4)
        kk = work.tile([128, B], F32, name=f"kk{ti}", tag="kk", bufs=3)
        nc.vector.tensor_scalar(out=kk, in0=uu, scalar1=MAGIC, scalar2=MAGIC,
                                op0=OP.add, op1=OP.subtract)
        ff = work.tile([128, B], F32, name=f"ff{ti}", tag="ff", bufs=3)
        nc.gpsimd.tensor_sub(ff, uu, kk)
        e = work.tile([128, B], BF16, name=f"enc{ti}", tag="enc", bufs=6)
        nc.scalar.activation(out=e, in_=ff, func=AF.Sin, scale=TWO_PI)
        enc_tiles[ti] = e

    emit_angle(0)
    emit_angle(1)
    load_consts([n for n in CONST_SPECS if n not in EARLY])

    # gather destinations, filled per-slab during the main loop
    D64 = pre.tile([S, B], F32, name="D64", tag="D64")
    THg = [pre.tile([S, B], F32, name=f"TH{cc}", tag=f"TH{cc}")
           for cc in range(3)]

    def emit_gathers(sidx):
        src = CT[sidx // 4]
        q = sidx % 4
        nc.sync.dma_start(out=D64[4 * sidx: 4 * sidx + 4],
                          in_=src[32 * q: 32 * q + 4])
        for cc in range(3):
            nc.sync.dma_start(
                out=THg[cc][4 * sidx: 4 * sidx + 4],
                in_=src[32 * q + 8 + 8 * cc: 32 * q + 12 + 8 * cc])

    # evac engine per (layer, half): ScalarE folds the bias (relu+scale+bias
    # in one ACTIVATE); VectorE halves take bias from PSUM ('mm': packed k=1
    # bias matmuls) or from the encoding rows ('enc': L0/L4 carry [w;b]).
    # Strict engine alternation: every layer's m0 half on ScalarE (folded
    # bias), m1 half on VectorE (bias from PSUM via packed k=1 matmuls, or
    # from the encoding rows for L0/L4). Consecutive psum-slot evacs then
    # ping-pong between the two PSUM-draining engines, keeping both busy.
    EVH = {}
    for _li in range(7):
        EVH[(_li, 0)] = ("act", "enc" if _li in (0, 4) else None)
        EVH[(_li, 1)] = ("vec", "enc" if _li in (0, 4) else "mm")
    BIAS_MM = {(li, 1): f"biasM1L{li}" for li in (1, 2, 3, 5, 6)}
    BIAS_MM[7] = "biasL7"

    def evac_half(li, m, pp_slot, hpair):
        # pp_slot [128, 2(chunk), 512] -> hpair[:, m, :, :]
        eng, bmode = EVH[(li, m)]
        out = hpair[:, m, :, :]
        if eng == "act":
            bias = 0.0 if bmode == "enc" else sb[f"b{li}sAS"][:, m:m + 1]
            nc.scalar.activation(out=out, in_=pp_slot[:, :, :B], func=AF.Relu,
                                 scale=INV_WS, bias=bias)
        else:
            nc.vector.tensor_scalar(out=out, in0=pp_slot[:, :, :B],
                                    scalar1=0.0, scalar2=INV_WS,
                                    op0=OP.max, op1=OP.mult)

    # ---- main loop: 8 super-groups of 8 chunks ----
    for g in range(8):
        if g < 7:
            emit_angle(2 * (g + 1))
            emit_angle(2 * (g + 1) + 1)
        encA, encB = enc_tiles[2 * g], enc_tiles[2 * g + 1]

        def enc_of(c):
            return (encA if c < 4 else encB), 32 * (c % 4)

        def new_hpair():
            return work.tile([128, 2, 2, B], FP8, name="hp", tag="hp", bufs=10)

        def slot():
            return psum.tile([128, 2, 512], F32, name="pp", tag="pp", bufs=4)

        # --- L0 / L4: k=31 enc matmuls (row-group packed); halves
        # interleaved per pair so consecutive slot evacs alternate engines ---
        def enc_layer(li, wname, h_in):
            hp = [new_hpair() for _ in range(4)]
            for pr in range(4):
                for m in range(2):
                    sl = slot()
                    for ci in range(2):
                        c = 2 * pr + ci
                        e, rb = enc_of(c)
                        nc.tensor.matmul(sl[:, ci, :B],
                                         sb[f"{wname}{m}"][rb:rb + 32],
                                         e[rb:rb + 32], start=True,
                                         stop=(h_in is None),
                                         tile_position=(rb, 0))
                    if h_in is not None:
                        for ci in range(2):
                            c = 2 * pr + ci
                            nc.tensor.matmul(
                                sl[:, ci, :B],
                                sb["w4h"][:, :, 128 * m:128 * m + 128],
                                h_in[c // 2][:, :, c % 2, :],
                                start=False, stop=True, perf_mode=DR)
                    evac_half(li, m, sl, hp[pr])
            return hp

        h_prev = enc_layer(0, "w0x", None)
        if taps is not None and g == 0:
            taps["enc0"] = encA
            taps["h0"] = h_prev[0]
        tap_h6 = taps is not None and g == 0

        # --- DR hidden layers: halves interleaved per pair (consecutive
        # slot evacs alternate ACT/DVE); bias matmuls for the 'mm' half are
        # batched 4-packed at the head of each 4-chunk sub-phase ---
        def dr_layer(li, h_in):
            hp = [new_hpair() for _ in range(4)]
            has_bias = (li, 1) in BIAS_MM
            for hg in range(2):
                bslots = {}
                if has_bias:
                    bl = sb[BIAS_MM[(li, 1)]]
                    for p in (2 * hg, 2 * hg + 1):
                        bslots[p] = slot()
                    for c4 in range(4):
                        c = 4 * hg + c4
                        r = 32 * (c % 4)
                        nc.tensor.matmul(bslots[c // 2][:, c % 2, :B],
                                         bl[r:r + 1], ones4[r:r + 1],
                                         start=True, stop=False,
                                         tile_position=(r, 0))
                for p in (2 * hg, 2 * hg + 1):
                    for m in range(2):
                        sl = bslots[p] if (has_bias and m == 1) else slot()
                        for ci in range(2):
                            c = 2 * p + ci
                            nc.tensor.matmul(
                                sl[:, ci, :B],
                                sb[f"wq{li}"][:, :, 128 * m:128 * m + 128],
                                h_in[c // 2][:, :, c % 2, :],
                                start=(not (has_bias and m == 1)), stop=True,
                                perf_mode=DR)
                        evac_half(li, m, sl, hp[p])
            return hp

        for li in (1, 2, 3):
            h_prev = dr_layer(li, h_prev)
        h_prev = enc_layer(4, "w4ex", h_prev)
        for li in (5, 6):
            h_prev = dr_layer(li, h_prev)

        if tap_h6:
            taps["h6"] = h_prev[0]

        # --- L7f: bias MMs batched per 4 chunks, then DR run; vec evac ---
        F1p = [work.tile([128, 2, B], BF16, name="F1p", tag="F1p", bufs=6)
               for _ in range(4)]
        bl = sb["biasL7"]
        for hg in range(2):
            slots = [slot(), slot()]
            for c4 in range(4):
                c = 4 * hg + c4
                r = 32 * (c % 4)
                nc.tensor.matmul(slots[c4 // 2][:, c % 2, :B],
                                 bl[r:r + 1], ones4[r:r + 1],
                                 start=True, stop=False, tile_position=(r, 0))
            for c4 in range(4):
                c = 4 * hg + c4
                nc.tensor.matmul(slots[c4 // 2][:, c % 2, :B], sb["w7f"],
                                 h_prev[c // 2][:, :, c % 2, :],
                                 start=False, stop=True, perf_mode=DR)
            for pr in range(2):
                p_idx = 2 * hg + pr
                if p_idx % 2 == 0:
                    nc.scalar.activation(out=F1p[p_idx],
                                         in_=slots[pr][:, :, :B],
                                         func=AF.Relu, scale=INV_WS)
                else:
                    nc.vector.tensor_scalar(out=F1p[p_idx],
                                            in0=slots[pr][:, :, :B],
                                            scalar1=0.0, scalar2=INV_WS,
                                            op0=OP.max, op1=OP.mult)

        # --- tail: density + colors accumulate per 4-chunk half-group in a
        # regular pool slot (bank 0), freeing psum for a 4-deep rotation ---
        for hg in range(2):
            T = slot()
            for c4 in range(4):
                c = 4 * hg + c4
                nc.tensor.matmul(T[0:32, 0, :B], sb["w7d8"][:, c4, :, :],
                                 h_prev[c // 2][:, :, c % 2, :],
                                 start=(c4 == 0), stop=False, perf_mode=DR,
                                 skip_group_check=True)
            for c4 in range(4):
                c = 4 * hg + c4
                nc.tensor.matmul(T[0:32, 0, :B], sb["w8f8"][:, c4, :],
                                 F1p[c // 2][:, c % 2, :],
                                 start=False, stop=(c4 == 3),
                                 skip_group_check=True)
            sidx = 2 * g + hg
            tgt = CT[sidx // 4]
            q = sidx % 4
            nc.scalar.activation(out=tgt[32 * q: 32 * q + 32],
                                 in_=T[0:32, 0, :B], func=AF.Copy,
                                 scale=INV_PS)
            emit_gathers(sidx)

    # ---- compositing (gathers already streamed during the main loop) ----
    SG = pre.tile([S, B], F32, name="SG", tag="SG")
    nc.scalar.activation(out=SG, in_=D64, func=AF.Relu, bias=sb["b7d64"],
                         scale=1.0)
    M64 = pre.tile([S, B], BF16, name="M64", tag="M64")
    nc.vector.tensor_mul(M64, SG, dists)
    pc2 = psum.tile([128, 2, 512], F32, name="pp", tag="pp", bufs=4)
    nc.tensor.matmul(pc2[0:S, 0, :B], sb["ltri"], M64, start=True, stop=True)
    nc.tensor.matmul(pc2[0:S, 1, :B], sb["ltri2"], M64, start=True, stop=True)
    T64 = pre.tile([S, B], F32, name="T64", tag="T64")
    nc.scalar.activation(out=T64, in_=pc2[0:S, 0, :B], func=AF.Exp, scale=-1.0)
    T64b = pre.tile([S, B], F32, name="T64b", tag="T64b")
    nc.scalar.activation(out=T64b, in_=pc2[0:S, 1, :B], func=AF.Exp, scale=-1.0)
    w64 = pre.tile([S, B], BF16, name="w64", tag="w64")
    nc.vector.tensor_sub(w64, T64, T64b)
    if taps is not None:
        taps["D64"] = D64
        taps["w64"] = w64
        taps["CTa"] = CT[0]
        taps["CTb"] = CT[1]

    for cc in range(3):
        Sc = pre.tile([S, B], F32, name=f"Sc{cc}", tag=f"Sc{cc}")
        nc.vector.tensor_add(Sc, THg[cc], vcb[cc])
        St = pre.tile([S, B], BF16, name=f"St{cc}", tag=f"St{cc}")
        nc.scalar.activation(out=St, in_=Sc, func=AF.Sigmoid, scale=WS,
                             bias=sb[f"b8c64_{cc}"])
        P = pre.tile([S, B], BF16, name=f"P{cc}", tag=f"P{cc}")
        nc.vector.tensor_mul(P, w64, St)
        pc = psum.tile([128, 2, 512], F32, name="pp", tag="pp", bufs=4)
        nc.tensor.matmul(pc[0:1, 0, :B], sb["ones641"], P, start=True, stop=True)
        oc = pre.tile([1, B], F32, name=f"oc{cc}", tag=f"oc{cc}")
        nc.scalar.activation(out=oc, in_=pc[0:1, 0, :B], func=AF.Copy)
        nc.sync.dma_start(out=out_ap.rearrange("b c -> c b")[cc:cc + 1], in_=oc)

    if taps is not None:
        for tname, t in taps.items():
            nc.sync.dma_start(out=a["tap_" + tname], in_=t)


TAP_SPECS = {
    "enc0": ([128, BL], BF16), "h0": ([128, 2, 2, BL], FP8),
    "D64": ([S, BL], F32), "w64": ([S, BL], BF16), "CTa": ([128, BL], F32),
    "CTb": ([128, BL], F32), "h6": ([128, 2, 2, BL], FP8),
}


def build_nc(debug_taps=False):
    nc = bacc.Bacc("TRN2", target_bir_lowering=False, debug=False)
    aps = {}
    for name, (shape, dt) in {**IN_SPECS, **CONST_SPECS}.items():
        aps[name] = nc.dram_tensor(name, list(shape), dt,
                                   kind="ExternalInput").ap()
    taps = None
    if debug_taps:
        taps = {}
        for tname, (shape, dt) in TAP_SPECS.items():
            aps["tap_" + tname] = nc.dram_tensor(
                "tap_" + tname, shape, dt, kind="ExternalOutput").ap()
    out = nc.dram_tensor("out", [BL, 3], F32, kind="ExternalOutput").ap()
    with tile.TileContext(nc) as tc, ExitStack() as ctx:
        build_nerf(tc, ctx, out, aps, taps=taps)
    nc.compile()
    return nc


def make_in_maps(inputs):
    consts = host_constants()
    wts = host_weights(inputs)
    shared = {**consts, **wts}
    for name, (shape, dt) in CONST_SPECS.items():
        v = np.ascontiguousarray(shared[name])
        assert v.shape == tuple(shape), (name, v.shape, shape)
        shared[name] = v
    in_maps = []
    for core in range(N_CORES):
        sl = slice(core * BL, (core + 1) * BL)
        m = dict(shared)
        m["xT"] = np.ascontiguousarray(np.asarray(inputs["x"])[sl].T,
                                       dtype=np.float32)
        m["off"] = np.ascontiguousarray(
            np.asarray(inputs["offsets"])[:, sl], dtype=np.float32)
        in_maps.append(m)
    return in_maps


def kernel(**inputs):
    from concourse.bass_utils import run_bass_kernel_spmd

    nc = build_nc()
    in_maps = make_in_maps(inputs)
    res = run_bass_kernel_spmd(nc, in_maps, core_ids=list(range(N_CORES)))
    out = np.concatenate([r["out"] for r in res.results], axis=0)
    return out.astype(np.float32)


# revision 69
# speedup vs baseline: 1.0499x; 1.0267x over previous
"""NeRF MLP kernel for Trainium2 (Bass/Tile), 8-core data-parallel over rays.

v3 design — layer-major, weight-stationary, evac-balanced:

- Layout: features on SBUF partitions, rays (512/core) on the free dim; one
  "chunk" = one sample index s for all local rays.  Chunks are processed in
  super-groups of 8, LAYER-major within the group, so each layer's weights
  stay stationary in the PE across 8 back-to-back matmuls (keeps the PE
  dense/warm; the HAM clock-gate stays at 8/8).
- Hidden matmuls are fp8e4 DoubleRow (weights host-scaled by 512,
  activations stored as 32*h): each 256-contraction layer half is one PE
  matmul at N=512.
- ALL biases are pre-accumulated into PSUM: L0/L4 carry [w;b] rows in the
  k=31 encoding matmuls (enc row 30 is sin(pi/2)=1); L1-L7f get packed k=1
  bias matmuls (rows at 32-aligned groups, even/odd chunks use different
  groups so adjacent bias matmuls pack concurrently).  Every PSUM->SBUF
  evacuation is then a single instruction: relu+scale on ScalarE or
  max0+mult on VectorE — the two PSUM-capable engines, which are the
  throughput floor of this kernel (~1 elem/cycle/partition each).
- Encoding uses the EXACT per-sample z (offsets do not cancel): angle
  u = (s+off[s])*DStep + Qb computed per 4-chunk tile on GpSimd (idle
  otherwise), magic-add range reduction on VectorE, one ScalarE Sin.
- Density (L7d) and color (w8f) rows accumulate IN PSUM across each
  super-group via one-hot weight columns — one cheap [32,512] evac per 8
  chunks instead of per-chunk copies.
- Compositing: w_s = exp(-cumsum) difference via bf16 triangular matmuls.
"""

import math
from contextlib import ExitStack

import numpy as np
import ml_dtypes

import concourse.bass as bass
import concourse.mybir as mybir
import concourse.tile as tile
from concourse import bacc

F32 = mybir.dt.float32
BF16 = mybir.dt.bfloat16
FP8 = mybir.dt.float8e4
AF = mybir.ActivationFunctionType
OP = mybir.AluOpType
DR = mybir.MatmulPerfMode.DoubleRow

S = 64          # samples per ray
B_FULL = 4096   # total rays
N_CORES = 8
BL = B_FULL // N_CORES  # rays per core = 512
H = 256
NEAR, FAR = 2.0, 6.0
DELTA = (FAR - NEAR) / S
L_ENC = 5
ENC = 3 * L_ENC * 2  # 30
TWO_PI = 2.0 * math.pi
MAGIC = 12582912.0  # 1.5 * 2**23, fp32 round-to-nearest trick

WS = 512.0      # fp8 weight scale
AS = 32.0       # fp8 activation scale (stored act = 32*h)
PS = WS * AS    # psum scale for biased layers = 16384
INV_WS = 1.0 / WS    # 2^-9: psum -> stored-act scale
INV_PS = 1.0 / PS    # 2^-14: tail evac scale

NPF8 = ml_dtypes.float8_e4m3
NPBF = ml_dtypes.bfloat16

# evac engine per hidden layer (7 = L7f). 'vec' layers use max0+mult
# tensor_scalar; 'act' layers use Relu activation. All read bias from PSUM.
EV = {0: "vec", 1: "vec", 2: "act", 3: "act", 4: "vec", 5: "act", 6: "act",
      7: "vec"}


def host_constants():
    c = {}
    freqs = (2.0 ** (np.arange(L_ENC, dtype=np.float64) - 2)) * math.pi  # [L]
    fturn = np.zeros((ENC, 1), dtype=np.float32)
    phase = np.zeros((ENC, 1), dtype=np.float32)
    for cc in range(3):
        for ll in range(L_ENC):
            for tt in range(2):
                j = cc * (L_ENC * 2) + ll * 2 + tt
                fturn[j, 0] = freqs[ll] / TWO_PI
                phase[j, 0] = 0.0 if tt == 0 else 0.25  # pi/2 in turns
    c["fturn30"] = fturn
    c["phase30"] = phase
    c["cap1e10"] = np.full((1, BL), 1.0e10, dtype=np.float32)
    c["q025"] = np.full((1, BL), 0.25, dtype=np.float32)
    c["svec64"] = (NEAR + np.arange(S, dtype=np.float32)[:, None] * DELTA)
    c["srow64"] = np.arange(S, dtype=np.float32)[:, None]
    c["ltri"] = np.triu(np.ones((S, S), dtype=np.float32)).astype(NPBF)
    c["ltri2"] = (np.triu(np.ones((S, S))) + np.eye(S)).astype(NPBF)
    c["ones31"] = np.ones((3, 1), dtype=np.float32)
    c["ones641"] = np.ones((S, 1), dtype=NPBF)
    return c


def host_weights(inp):
    w = {}

    def kstack(m):  # [256, M] -> [128, 2, M]
        return np.ascontiguousarray(m.reshape(2, 128, m.shape[1]).transpose(1, 0, 2))

    # k=31 encoding matmuls carry weight+bias scaled by PS: 4 identical
    # 32-row blocks (one per chunk slot in a 4-chunk enc tile); row 30 is
    # the bias row (enc row 30 evaluates to 1), row 31 zero.
    def blk4(wmat, bvec, m):
        t = np.zeros((128, 128), dtype=np.float32)
        for j in range(4):
            t[32 * j: 32 * j + ENC] = wmat[:, m * 128:(m + 1) * 128] * PS
            t[32 * j + ENC] = bvec[m * 128:(m + 1) * 128] * PS
        return t.astype(NPBF)

    for m in range(2):
        w[f"w0x{m}"] = blk4(inp["w0"], inp["b0"], m)
        w[f"w4ex{m}"] = blk4(inp["w4"][H:H + ENC], inp["b4"], m)

    # fp8 DoubleRow weights, scaled by WS
    for i in (1, 2, 3, 5, 6):
        w[f"wq{i}"] = (kstack(inp[f"w{i}"]) * WS).astype(NPF8)
    w["w4h"] = (kstack(inp["w4"][0:H]) * WS).astype(NPF8)
    w["w7f"] = (kstack(inp["w7"][:, 1:129]) * WS).astype(NPF8)

    # ACT-folded biases: AS*b per half as per-partition columns
    for i in (1, 2, 3, 5, 6):
        w[f"b{i}sAS"] = np.ascontiguousarray(
            inp[f"b{i}"].reshape(2, 128).T * AS).astype(np.float32)  # [128,2]

    # k=1 bias-matmul rows (PS-scaled) for the VecE-evac'd halves: the same
    # bias row duplicated at all four 32-row groups so the bias MMs of 4
    # consecutive chunks pack concurrently (rows 32*(c%4)).
    def bias4(vec):
        t = np.zeros((128, 128), dtype=np.float32)
        for j in range(4):
            t[32 * j] = vec * PS
        return t.astype(NPBF)

    for i in (1, 2, 3, 5, 6):
        w[f"biasM1L{i}"] = bias4(inp[f"b{i}"][128:256])
    w["biasL7"] = bias4(inp["b7"][1:129])

    # tail: density one-hot col i (of 8), DR fp8, w7[:,0]*WS. Cols padded to
    # 32 so the c==0 start=True matmul covers (and clears has_written for)
    # the FULL 32-partition tail region each group -- the color matmuls
    # (start=False) then overwrite/accumulate correctly; without this the
    # color partitions 16-31 keep stale accumulation across bank reuse.
    wd = np.zeros((128, 8, 2, 32), dtype=np.float32)
    for i in range(8):
        pad = np.zeros((256, 32), dtype=np.float32)
        pad[:, i] = inp["w7"][:, 0] * WS
        wd[:, i] = kstack(pad)
    w["w7d8"] = wd.astype(NPF8)
    # tail: color one-hot col 8 + 8c + i (c-plane-major for contiguous
    # gathers), bf16, unscaled (F1 carries the 32x)
    wc = np.zeros((128, 8, 32), dtype=np.float32)
    for i in range(8):
        for cc in range(3):
            wc[:, i, 8 + 8 * cc + i] = inp["w8"][0:128, cc]
    w["w8f8"] = wc.astype(NPBF)

    w["w8v3s"] = (inp["w8"][128:131] / WS).astype(np.float32)  # [3,3]
    w["b7d64"] = np.full((S, 1), inp["b7"][0], dtype=np.float32)
    for cc in range(3):
        w[f"b8c64_{cc}"] = np.full((S, 1), inp["b8"][cc], dtype=np.float32)
    return w


CONST_SPECS = {
    "w0x0": ((128, 128), BF16), "w0x1": ((128, 128), BF16),
    "w4ex0": ((128, 128), BF16), "w4ex1": ((128, 128), BF16),
    "wq1": ((128, 2, 256), FP8), "wq2": ((128, 2, 256), FP8),
    "wq3": ((128, 2, 256), FP8), "w4h": ((128, 2, 256), FP8),
    "wq5": ((128, 2, 256), FP8), "wq6": ((128, 2, 256), FP8),
    "w7f": ((128, 2, 128), FP8),
    "b1sAS": ((128, 2), F32), "b2sAS": ((128, 2), F32),
    "b3sAS": ((128, 2), F32), "b5sAS": ((128, 2), F32),
    "b6sAS": ((128, 2), F32),
    "biasM1L1": ((128, 128), BF16), "biasM1L2": ((128, 128), BF16),
    "biasM1L3": ((128, 128), BF16), "biasM1L5": ((128, 128), BF16),
    "biasM1L6": ((128, 128), BF16), "biasL7": ((128, 128), BF16),
    "w7d8": ((128, 8, 2, 32), FP8),
    "w8f8": ((128, 8, 32), BF16),
    "w8v3s": ((3, 3), F32),
    "b7d64": ((S, 1), F32),
    "b8c64_0": ((S, 1), F32), "b8c64_1": ((S, 1), F32), "b8c64_2": ((S, 1), F32),
    "fturn30": ((ENC, 1), F32), "phase30": ((ENC, 1), F32),
    "svec64": ((S, 1), F32), "srow64": ((S, 1), F32),
    "cap1e10": ((1, BL), F32), "q025": ((1, BL), F32),
    "ltri": ((S, S), BF16), "ltri2": ((S, S), BF16),
    "ones31": ((3, 1), F32), "ones641": ((S, 1), BF16),
}

IN_SPECS = {"xT": ((6, BL), F32), "off": ((S, BL), F32)}


def bcast_rows(ap, reps, cols):
    rows = ap.shape[0]
    return bass.AP(
        tensor=ap.tensor,
        offset=ap.offset,
        ap=[[ap.ap[0][0], rows], [0, reps], [1, cols]],
    )


def build_nerf(tc, ctx, out_ap, a, taps=None):
    nc = tc.nc
    B = BL

    consts = ctx.enter_context(tc.tile_pool(name="consts", bufs=1))
    pre = ctx.enter_context(tc.tile_pool(name="pre", bufs=1))
    work = ctx.enter_context(tc.tile_pool(name="work", bufs=2))
    psum = ctx.enter_context(tc.tile_pool(name="psum", bufs=1, space="PSUM"))

    # ---- constants / weights straight into SBUF (host pre-cast) ----
    # Small early constants (angle path, pre-phase, L0) are DMA'd first;
    # the bulky hidden-layer weights are deferred until after the pre-phase
    # and the first angle tiles are emitted, so they don't sit in front of
    # the latency-critical startup DMAs in the queue.
    EARLY = ("fturn30", "phase30", "srow64", "svec64", "cap1e10", "q025",
             "ones31", "w8v3s", "w0x0", "w0x1", "b7d64",
             "b8c64_0", "b8c64_1", "b8c64_2", "ltri", "ltri2", "ones641")
    sb = {}

    def load_consts(names, eng=None):
        # const loads can ride any engine's queue; the late (bulky) weight
        # batch goes via ScalarE, which is idle during the ramp, so the
        # Sync queue stays free for the latency-critical angle broadcasts.
        eng = eng or nc.sync
        for name in names:
            shape, dt = CONST_SPECS[name]
            t = consts.tile(list(shape), dt, name=name, tag=name)
            eng.dma_start(out=t, in_=a[name])
            sb[name] = t

    load_consts(EARLY)
    ones4 = consts.tile([128, B], BF16, name="ones4", tag="ones4")
    nc.vector.memset(ones4, 1.0)

    # ---- pre-phase ----
    dt3 = pre.tile([3, B], F32, name="dt3", tag="dt3")
    nc.sync.dma_start(out=dt3, in_=a["xT"][3:6])
    off = pre.tile([S, B], F32, name="off", tag="off")
    nc.sync.dma_start(out=off, in_=a["off"])

    D30 = pre.tile([ENC, B], F32, name="D30", tag="D30")
    nc.sync.dma_start(out=D30, in_=bcast_rows(a["xT"][3:6], 2 * L_ENC, B))
    O30 = pre.tile([ENC, B], F32, name="O30", tag="O30")
    nc.sync.dma_start(out=O30, in_=bcast_rows(a["xT"][0:3], 2 * L_ENC, B))
    DF = pre.tile([ENC, B], F32, name="DF", tag="DF")
    nc.vector.tensor_scalar(out=DF, in0=D30, scalar1=sb["fturn30"],
                            scalar2=None, op0=OP.mult)
    AO = pre.tile([ENC, B], F32, name="AO", tag="AO")
    nc.vector.tensor_scalar(out=AO, in0=O30, scalar1=sb["fturn30"],
                            scalar2=sb["phase30"], op0=OP.mult, op1=OP.add)
    DStep30 = pre.tile([ENC, B], F32, name="DStep30", tag="DStep30")
    nc.vector.tensor_scalar(out=DStep30, in0=DF, scalar1=float(DELTA),
                            scalar2=None, op0=OP.mult)
    Qb30 = pre.tile([ENC, B], F32, name="Qb30", tag="Qb30")
    nc.vector.affine_then_add(out=Qb30, in0=DF, in1=AO, scale=float(NEAR),
                              bias=0.0)

    # assemble 4-block [128,B] versions; rows 30 of each block: DStep=0,
    # Qb=0.25 (bias row -> sin=1); rows 31: 0 (zero pad).
    DStep4 = pre.tile([128, B], F32, name="DStep4", tag="DStep4")
    nc.vector.memset(DStep4, 0.0)
    Qb4 = pre.tile([128, B], F32, name="Qb4", tag="Qb4")
    nc.vector.memset(Qb4, 0.0)
    for j in range(4):
        nc.sync.dma_start(out=DStep4[32 * j: 32 * j + ENC], in_=DStep30)
        nc.sync.dma_start(out=Qb4[32 * j: 32 * j + ENC], in_=Qb30)
        nc.sync.dma_start(out=Qb4[32 * j + ENC: 32 * j + ENC + 1],
                          in_=a["q025"])

    off_plus = pre.tile([S, B], F32, name="off_plus", tag="off_plus")
    nc.vector.tensor_scalar(out=off_plus, in0=off, scalar1=sb["srow64"],
                            scalar2=None, op0=OP.add)

    # |d| and view-dir color contribution
    sq3 = pre.tile([3, B], F32, name="sq3", tag="sq3")
    nc.vector.tensor_mul(sq3, dt3, dt3)
    p0 = psum.tile([128, 2, 512], F32, name="pp", tag="pp", bufs=4)
    nc.tensor.matmul(p0[0:1, 0, :B], sb["ones31"], sq3, start=True, stop=True)
    nd = pre.tile([1, B], F32, name="nd", tag="nd")
    nc.scalar.activation(out=nd, in_=p0[0:1, 0, :B], func=AF.Sqrt)
    inv_nd = pre.tile([1, B], F32, name="inv_nd", tag="inv_nd")
    nc.vector.reciprocal(out=inv_nd, in_=nd)
    inv3 = pre.tile([3, B], F32, name="inv3", tag="inv3")
    nc.gpsimd.partition_broadcast(inv3, inv_nd)
    v3 = pre.tile([3, B], F32, name="v3", tag="v3")
    nc.vector.tensor_mul(v3, dt3, inv3)
    p1 = psum.tile([128, 2, 512], F32, name="pp", tag="pp", bufs=4)
    nc.tensor.matmul(p1[0:3, 0, :B], sb["w8v3s"], v3, start=True, stop=True)
    vc3 = pre.tile([3, B], F32, name="vc3", tag="vc3")
    nc.scalar.activation(out=vc3, in_=p1[0:3, 0, :B], func=AF.Copy)
    vcb = []
    for cc in range(3):
        t = pre.tile([S, B], F32, name=f"vcb{cc}", tag=f"vcb{cc}")
        nc.sync.dma_start(out=t, in_=bcast_rows(vc3[cc:cc + 1], S, B))
        vcb.append(t)

    # dists
    Z = pre.tile([S, B], F32, name="Z", tag="Z")
    nc.vector.tensor_scalar(out=Z, in0=off, scalar1=float(DELTA),
                            scalar2=sb["svec64"], op0=OP.mult, op1=OP.add)
    nd64 = pre.tile([S, B], F32, name="nd64", tag="nd64")
    nc.gpsimd.partition_broadcast(nd64, nd)
    ZN = pre.tile([S, B], F32, name="ZN", tag="ZN")
    nc.vector.tensor_mul(ZN, Z, nd64)
    ZNs = pre.tile([S, B], F32, name="ZNs", tag="ZNs")
    nc.sync.dma_start(out=ZNs[0: S - 1], in_=ZN[1:S])
    nc.sync.dma_start(out=ZNs[S - 1: S], in_=a["cap1e10"])
    dists = pre.tile([S, B], F32, name="dists", tag="dists")
    nc.vector.tensor_sub(dists, ZNs, ZN)

    # CT destination slabs ([32,B] per 4-chunk half-group, 16 slabs)
    CT = [pre.tile([128, B], F32, name=f"CT{i}", tag=f"CT{i}")
          for i in range(4)]

    # ---- angle tiles (4 chunks each) ----
    enc_tiles = {}

    def emit_angle(ti):
        OFF4 = work.tile([128, B], F32, name=f"off4_{ti}", tag="off4", bufs=4)
        for j in range(4):
            s = 4 * ti + j
            nc.gpsimd.dma_start(out=OFF4[32 * j: 32 * j + 32],
                                in_=bcast_rows(off_plus[s:s + 1], 32, B))
        um = work.tile([128, B], F32, name=f"um{ti}", tag="um", bufs=3)
        nc.gpsimd.tensor_mul(um, OFF4, DStep4)
        uu = work.tile([128, B], F32, name=f"uu{ti}", tag="uu", bufs=3)
        nc.gpsimd.tensor_add(uu, um, Qb4)
        kk = work.tile([128, B], F32, name=f"kk{ti}", tag="kk", bufs=3)
        nc.vector.tensor_scalar(out=kk, in0=uu, scalar1=MAGIC, scalar2=MAGIC,
                                op0=OP.add, op1=OP.subtract)
        ff = work.tile([128, B], F32, name=f"ff{ti}", tag="ff", bufs=3)
        nc.gpsimd.tensor_sub(ff, uu, kk)
        e = work.tile([128, B], BF16, name=f"enc{ti}", tag="enc", bufs=6)
        nc.scalar.activation(out=e, in_=ff, func=AF.Sin, scale=TWO_PI)
        enc_tiles[ti] = e

    emit_angle(0)
    emit_angle(1)
    load_consts([n for n in CONST_SPECS if n not in EARLY], eng=nc.scalar)

    # gather destinations, filled per-slab during the main loop
    D64 = pre.tile([S, B], F32, name="D64", tag="D64")
    THg = [pre.tile([S, B], F32, name=f"TH{cc}", tag=f"TH{cc}")
           for cc in range(3)]

    def emit_gathers(sidx):
        src = CT[sidx // 4]
        q = sidx % 4
        nc.sync.dma_start(out=D64[4 * sidx: 4 * sidx + 4],
                          in_=src[32 * q: 32 * q + 4])
        for cc in range(3):
            nc.sync.dma_start(
                out=THg[cc][4 * sidx: 4 * sidx + 4],
                in_=src[32 * q + 8 + 8 * cc: 32 * q + 12 + 8 * cc])

    # evac engine per (layer, half): ScalarE folds the bias (relu+scale+bias
    # in one ACTIVATE); VectorE halves take bias from PSUM ('mm': packed k=1
    # bias matmuls) or from the encoding rows ('enc': L0/L4 carry [w;b]).
    # Strict engine alternation: every layer's m0 half on ScalarE (folded
    # bias), m1 half on VectorE (bias from PSUM via packed k=1 matmuls, or
    # from the encoding rows for L0/L4). Consecutive psum-slot evacs then
    # ping-pong between the two PSUM-draining engines, keeping both busy.
    EVH = {}
    for _li in range(7):
        EVH[(_li, 0)] = ("act", "enc" if _li in (0, 4) else None)
        EVH[(_li, 1)] = ("vec", "enc" if _li in (0, 4) else "mm")
    BIAS_MM = {(li, 1): f"biasM1L{li}" for li in (1, 2, 3, 5, 6)}
    BIAS_MM[7] = "biasL7"

    def evac_half(li, m, pp_slot, hpair):
        # pp_slot [128, 2(chunk), 512] -> hpair[:, m, :, :]
        eng, bmode = EVH[(li, m)]
        out = hpair[:, m, :, :]
        if eng == "act":
            bias = 0.0 if bmode == "enc" else sb[f"b{li}sAS"][:, m:m + 1]
            nc.scalar.activation(out=out, in_=pp_slot[:, :, :B], func=AF.Relu,
                                 scale=INV_WS, bias=bias)
        else:
            nc.vector.tensor_scalar(out=out, in0=pp_slot[:, :, :B],
                                    scalar1=0.0, scalar2=INV_WS,
                                    op0=OP.max, op1=OP.mult)

    # ---- main loop: 8 super-groups of 8 chunks ----
    for g in range(8):
        if g < 7:
            emit_angle(2 * (g + 1))
            emit_angle(2 * (g + 1) + 1)
        encA, encB = enc_tiles[2 * g], enc_tiles[2 * g + 1]

        def enc_of(c):
            return (encA if c < 4 else encB), 32 * (c % 4)

        def new_hpair():
            return work.tile([128, 2, 2, B], FP8, name="hp", tag="hp", bufs=10)

        def slot():
            return psum.tile([128, 2, 512], F32, name="pp", tag="pp", bufs=4)

        # --- L0 / L4: k=31 enc matmuls (row-group packed); halves
        # interleaved per pair so consecutive slot evacs alternate engines ---
        def enc_layer(li, wname, h_in):
            hp = [new_hpair() for _ in range(4)]
            for pr in range(4):
                for m in range(2):
                    sl = slot()
                    for ci in range(2):
                        c = 2 * pr + ci
                        e, rb = enc_of(c)
                        nc.tensor.matmul(sl[:, ci, :B],
                                         sb[f"{wname}{m}"][rb:rb + 32],
                                         e[rb:rb + 32], start=True,
                                         stop=(h_in is None),
                                         tile_position=(rb, 0))
                    if h_in is not None:
                        for ci in range(2):
                            c = 2 * pr + ci
                            nc.tensor.matmul(
                                sl[:, ci, :B],
                                sb["w4h"][:, :, 128 * m:128 * m + 128],
                                h_in[c // 2][:, :, c % 2, :],
                                start=False, stop=True, perf_mode=DR)
                    evac_half(li, m, sl, hp[pr])
            return hp

        h_prev = enc_layer(0, "w0x", None)
        if taps is not None and g == 0:
            taps["enc0"] = encA
            taps["h0"] = h_prev[0]
        tap_h6 = taps is not None and g == 0

        # --- DR hidden layers: halves interleaved per pair (consecutive
        # slot evacs alternate ACT/DVE); bias matmuls for the 'mm' half are
        # batched 4-packed at the head of each 4-chunk sub-phase ---
        def dr_layer(li, h_in):
            hp = [new_hpair() for _ in range(4)]
            has_bias = (li, 1) in BIAS_MM
            for hg in range(2):
                bslots = {}
                if has_bias:
                    bl = sb[BIAS_MM[(li, 1)]]
                    for p in (2 * hg, 2 * hg + 1):
                        bslots[p] = slot()
                    for c4 in range(4):
                        c = 4 * hg + c4
                        r = 32 * (c % 4)
                        nc.tensor.matmul(bslots[c // 2][:, c % 2, :B],
                                         bl[r:r + 1], ones4[r:r + 1],
                                         start=True, stop=False,
                                         tile_position=(r, 0))
                for p in (2 * hg, 2 * hg + 1):
                    for m in range(2):
                        sl = bslots[p] if (has_bias and m == 1) else slot()
                        for ci in range(2):
                            c = 2 * p + ci
                            nc.tensor.matmul(
                                sl[:, ci, :B],
                                sb[f"wq{li}"][:, :, 128 * m:128 * m + 128],
                                h_in[c // 2][:, :, c % 2, :],
                                start=(not (has_bias and m == 1)), stop=True,
                                perf_mode=DR)
                        evac_half(li, m, sl, hp[p])
            return hp

        for li in (1, 2, 3):
            h_prev = dr_layer(li, h_prev)
        h_prev = enc_layer(4, "w4ex", h_prev)
        for li in (5, 6):
            h_prev = dr_layer(li, h_prev)

        if tap_h6:
            taps["h6"] = h_prev[0]

        # --- L7f: bias MMs batched per 4 chunks, then DR run; vec evac ---
        F1p = [work.tile([128, 2, B], BF16, name="F1p", tag="F1p", bufs=6)
               for _ in range(4)]
        bl = sb["biasL7"]
        for hg in range(2):
            slots = [slot(), slot()]
            for c4 in range(4):
                c = 4 * hg + c4
                r = 32 * (c % 4)
                nc.tensor.matmul(slots[c4 // 2][:, c % 2, :B],
                                 bl[r:r + 1], ones4[r:r + 1],
                                 start=True, stop=False, tile_position=(r, 0))
            for c4 in range(4):
                c = 4 * hg + c4
                nc.tensor.matmul(slots[c4 // 2][:, c % 2, :B], sb["w7f"],
                                 h_prev[c // 2][:, :, c % 2, :],
                                 start=False, stop=True, perf_mode=DR)
            for pr in range(2):
                p_idx = 2 * hg + pr
                if p_idx % 2 == 0:
                    nc.scalar.activation(out=F1p[p_idx],
                                         in_=slots[pr][:, :, :B],
                                         func=AF.Relu, scale=INV_WS)
                else:
                    nc.vector.tensor_scalar(out=F1p[p_idx],
                                            in0=slots[pr][:, :, :B],
                                            scalar1=0.0, scalar2=INV_WS,
                                            op0=OP.max, op1=OP.mult)

        # --- tail: density + colors accumulate per 4-chunk half-group in a
        # regular pool slot (bank 0), freeing psum for a 4-deep rotation ---
        for hg in range(2):
            T = slot()
            for c4 in range(4):
                c = 4 * hg + c4
                nc.tensor.matmul(T[0:32, 0, :B], sb["w7d8"][:, c4, :, :],
                                 h_prev[c // 2][:, :, c % 2, :],
                                 start=(c4 == 0), stop=False, perf_mode=DR,
                                 skip_group_check=True)
            for c4 in range(4):
                c = 4 * hg + c4
                nc.tensor.matmul(T[0:32, 0, :B], sb["w8f8"][:, c4, :],
                                 F1p[c // 2][:, c % 2, :],
                                 start=False, stop=(c4 == 3),
                                 skip_group_check=True)
            sidx = 2 * g + hg
            tgt = CT[sidx // 4]
            q = sidx % 4
            nc.scalar.activation(out=tgt[32 * q: 32 * q + 32],
                                 in_=T[0:32, 0, :B], func=AF.Copy,
                                 scale=INV_PS)
            emit_gathers(sidx)

    # ---- compositing (gathers already streamed during the main loop) ----
    SG = pre.tile([S, B], F32, name="SG", tag="SG")
    nc.scalar.activation(out=SG, in_=D64, func=AF.Relu, bias=sb["b7d64"],
                         scale=1.0)
    M64 = pre.tile([S, B], BF16, name="M64", tag="M64")
    nc.vector.tensor_mul(M64, SG, dists)
    pc2 = psum.tile([128, 2, 512], F32, name="pp", tag="pp", bufs=4)
    nc.tensor.matmul(pc2[0:S, 0, :B], sb["ltri"], M64, start=True, stop=True)
    nc.tensor.matmul(pc2[0:S, 1, :B], sb["ltri2"], M64, start=True, stop=True)
    T64 = pre.tile([S, B], F32, name="T64", tag="T64")
    nc.scalar.activation(out=T64, in_=pc2[0:S, 0, :B], func=AF.Exp, scale=-1.0)
    T64b = pre.tile([S, B], F32, name="T64b", tag="T64b")
    nc.scalar.activation(out=T64b, in_=pc2[0:S, 1, :B], func=AF.Exp, scale=-1.0)
    w64 = pre.tile([S, B], BF16, name="w64", tag="w64")
    nc.vector.tensor_sub(w64, T64, T64b)
    if taps is not None:
        taps["D64"] = D64
        taps["w64"] = w64
        taps["CTa"] = CT[0]
        taps["CTb"] = CT[1]

    for cc in range(3):
        Sc = pre.tile([S, B], F32, name=f"Sc{cc}", tag=f"Sc{cc}")
        nc.vector.tensor_add(Sc, THg[cc], vcb[cc])
        St = pre.tile([S, B], BF16, name=f"St{cc}", tag=f"St{cc}")
        nc.scalar.activation(out=St, in_=Sc, func=AF.Sigmoid, scale=WS,
                             bias=sb[f"b8c64_{cc}"])
        P = pre.tile([S, B], BF16, name=f"P{cc}", tag=f"P{cc}")
        nc.vector.tensor_mul(P, w64, St)
        pc = psum.tile([128, 2, 512], F32, name="pp", tag="pp", bufs=4)
        nc.tensor.matmul(pc[0:1, 0, :B], sb["ones641"], P, start=True, stop=True)
        oc = pre.tile([1, B], F32, name=f"oc{cc}", tag=f"oc{cc}")
        nc.scalar.activation(out=oc, in_=pc[0:1, 0, :B], func=AF.Copy)
        nc.sync.dma_start(out=out_ap.rearrange("b c -> c b")[cc:cc + 1], in_=oc)

    if taps is not None:
        for tname, t in taps.items():
            nc.sync.dma_start(out=a["tap_" + tname], in_=t)


TAP_SPECS = {
    "enc0": ([128, BL], BF16), "h0": ([128, 2, 2, BL], FP8),
    "D64": ([S, BL], F32), "w64": ([S, BL], BF16), "CTa": ([128, BL], F32),
    "CTb": ([128, BL], F32), "h6": ([128, 2, 2, BL], FP8),
}


def build_nc(debug_taps=False):
    nc = bacc.Bacc("TRN2", target_bir_lowering=False, debug=False)
    aps = {}
    for name, (shape, dt) in {**IN_SPECS, **CONST_SPECS}.items():
        aps[name] = nc.dram_tensor(name, list(shape), dt,
                                   kind="ExternalInput").ap()
    taps = None
    if debug_taps:
        taps = {}
        for tname, (shape, dt) in TAP_SPECS.items():
            aps["tap_" + tname] = nc.dram_tensor(
                "tap_" + tname, shape, dt, kind="ExternalOutput").ap()
    out = nc.dram_tensor("out", [BL, 3], F32, kind="ExternalOutput").ap()
    with tile.TileContext(nc) as tc, ExitStack() as ctx:
        build_nerf(tc, ctx, out, aps, taps=taps)
    nc.compile()
    return nc


def make_in_maps(inputs):
    consts = host_constants()
    wts = host_weights(inputs)
    shared = {**consts, **wts}
    for name, (shape, dt) in CONST_SPECS.items():
        v = np.ascontiguousarray(shared[name])
        assert v.shape == tuple(shape), (name, v.shape, shape)
        shared[name] = v
    in_maps = []
    for core in range(N_CORES):
        sl = slice(core * BL, (core + 1) * BL)
        m = dict(shared)
        m["xT"] = np.ascontiguousarray(np.asarray(inputs["x"])[sl].T,
                                       dtype=np.float32)
        m["off"] = np.ascontiguousarray(
            np.asarray(inputs["offsets"])[:, sl], dtype=np.float32)
        in_maps.append(m)
    return in_maps


def kernel(**inputs):
    from concourse.bass_utils import run_bass_kernel_spmd

    nc = build_nc()
    in_maps = make_in_maps(inputs)
    res = run_bass_kernel_spmd(nc, in_maps, core_ids=list(range(N_CORES)))
    out = np.concatenate([r["out"] for r in res.results], axis=0)
    return out.astype(np.float32)
